# revision 18
# baseline (speedup 1.0000x reference)
"""Trainium2 kernel for nn_LAM_Module_19052474925494.

Reference computation (B,N,C,H,W = 16,10,128,48,48):
  q = k = x.reshape(B,N,D), D = C*H*W = 294912
  s0 = (1-pd)*k[n] + pd*k[n+1]        (indices mod N)
  s1 = ld*((1-pd)*k[n+1] + pd*k[n+2])
  logits = [q.s0, q.s1]; attn = softmax(logits); out = attn0*s0 + attn1*s1
  feat = out.reshape(B, N*C, H, W)
  result = conv1x1(conv_w, feat) + conv_b + x.reshape(B, N*C, H, W)

Key numeric fact exploited: logit0 - logit1 ~ 1.5e5 >> 88 for iid N(0,1)
inputs of this size, so the fp32 softmax saturates *exactly* to attn = [1, 0]
(exp(-1.5e5) underflows to 0). Hence feat_n = (1-pd_n)*x_n + pd_n*x_{n+1},
linear in x, foldable into the conv weights host-side:

  result[b] = W_eff @ X_b + bias + X_b,   X_b = x[b] as [N*C, H*W]

A host-side guard computes the actual logit gaps and falls back to
materializing feat with the true attention weights when not saturated; the
device kernel is identical in both cases (residual always added on host).

Device kernel: one [1280 x 1280] @ [1280 x 4608] matmul per core (the two
batch items of this core side by side), data-parallel over batch across 8
cores, no collectives. Mixed precision on the contraction (K) dim:
  - K rows [0 : FP8_ROWS) in fp8-e4m3 with MatmulPerfMode.DoubleRow
    (2 MACs/cell/cycle, 0.5 cycles/row -> 2x PE throughput),
  - K rows [FP8_ROWS : 1280) in fp16 (1 cycle/row),
accumulated into the same fp32 PSUM bank. FP8_ROWS=512 measures rel err
~1.6e-2 end to end (budget 2e-2); the residual +X is exact (host fp32).
All inputs are resident in SBUF (~91 KiB/partition), outputs stream back
as fp16 and are upcast + residual-added on the host.
"""

import numpy as np

B, N, C, H, W = 16, 10, 128, 48, 48
NCh = N * C       # 1280 channels
HW = H * W        # 2304 spatial
NCORES = 8
BB = B // NCORES  # batch items per core
COLS = BB * HW    # 4608 moving columns per core (both items side by side)

# Tunables (test.py may override before the first kernel() call)
FP8_ROWS = 768    # K rows computed in fp8 DoubleRow; multiple of 256; 0 = off
NT = 512          # moving-column tile width (PSUM bank = 512 fp32)
OUT_DTYPE = "f16"  # "f16" or "f32" device output
PS_BUFS = 8
OSB_BUFS = 12  # >= 10: one paired output tile per ob is held across a ct sweep
X_BUFS = 3    # per-K-part X tile ring: ct, ct+1, ct+2 in flight
WARMUP_MMS = 0    # dependency-free PE warmup matmuls at start
OUT_ENGS = ("scalar", "gpsimd", "sync")  # engines issuing output DMAs (round-robin)
TRACE = False
TRACE_CORES = None
LAST_RESULT = None

_cache = {}


def _build_nc(fp8_rows):
    import concourse.bacc as bacc
    import concourse.mybir as mybir
    from concourse.tile import TileContext

    f32 = mybir.dt.float32
    f16 = mybir.dt.float16
    f8 = mybir.dt.float8e4
    out_dt = f16 if OUT_DTYPE == "f16" else f32
    DR = mybir.MatmulPerfMode.DoubleRow

    ndr = fp8_rows // 256
    nkb = (NCh - fp8_rows) // 128
    nct = COLS // NT

    nc = bacc.Bacc(None, target_bir_lowering=False, debug=False)
    # xs8p/xs16p are ct-major: [kpart * nct, C, ...] so each per-(kpart, ct)
    # tile load is fully contiguous per partition (1 KiB runs). x0p8/x0p16
    # pack ALL K-parts' ct0 slices into one tensor each (3-7 KiB/partition
    # contiguous) so the first matmul's data lands in two fast DMAs.
    xs8 = (
        nc.dram_tensor("xs8", [ndr * nct, C, 2, NT], f8, kind="ExternalInput")
        if ndr
        else None
    )
    xs16 = (
        nc.dram_tensor("xs16", [nkb * nct, C, NT], f16, kind="ExternalInput")
        if nkb
        else None
    )
    x0p8 = (
        nc.dram_tensor("x0p8", [C, ndr, 2, NT], f8, kind="ExternalInput")
        if ndr
        else None
    )
    x0p16 = (
        nc.dram_tensor("x0p16", [C, nkb, NT], f16, kind="ExternalInput")
        if nkb
        else None
    )
    wt8 = (
        nc.dram_tensor("wt8", [ndr, C, 2, NCh], f8, kind="ExternalInput")
        if ndr
        else None
    )
    wt16 = (
        nc.dram_tensor("wt16", [nkb, C, NCh], f16, kind="ExternalInput")
        if nkb
        else None
    )
    bias = nc.dram_tensor("bias", [C, N], f32, kind="ExternalInput")
    out = nc.dram_tensor("out", [NCh, COLS], out_dt, kind="ExternalOutput")

    with TileContext(nc) as tc:
        with (
            tc.tile_pool(name="wtp", bufs=1) as wt_pool,
            tc.tile_pool(name="biasp", bufs=1) as bias_pool,
            tc.tile_pool(name="xp", bufs=1) as x_pool,
            tc.tile_pool(name="psp", bufs=PS_BUFS, space="PSUM") as psum_pool,
            tc.tile_pool(name="op", bufs=OSB_BUFS) as out_pool,
        ):
            bias_sb = bias_pool.tile([C, N], f32, name="bias_sb")
            nc.scalar.dma_start(out=bias_sb[:], in_=bias[:])

            if WARMUP_MMS:
                wsc = bias_pool.tile([C, 128], f16, name="warm_sc")
                nc.vector.memset(wsc[:], 0.0)
                wps = psum_pool.tile([C, NT], f32, tag="ps", name="warm_ps")
                for _ in range(WARMUP_MMS):
                    nc.tensor.matmul(
                        wps[:, :128], wsc[:], wsc[:], start=True, stop=True
                    )

            wt8_sb = [None] * max(ndr, 1)
            wt16_sb = [None] * max(nkb, 1)
            x8_sb = {}
            x16_sb = {}

            # Tile deps are tile-granular, so X stays one tile per (ct,
            # K-part). The ct0 working set (weights + ct0 X) is spread over
            # the three DMA-capable engines in consumption order so the PE
            # can start ~8us in; later cts stream behind it round-robin.
            engs = [nc.sync, nc.scalar, nc.gpsimd]

            def load_x8(ct, t, e):
                tl = x_pool.tile(
                    [C, 2, NT], f8, tag=f"x8_{t}", bufs=X_BUFS, name=f"x8_{ct}_{t}"
                )
                e.dma_start(out=tl[:], in_=xs8[t * nct + ct])
                x8_sb[(ct, t)] = tl

            def load_x16(ct, kb, e):
                tl = x_pool.tile(
                    [C, NT], f16, tag=f"x16_{kb}", bufs=X_BUFS, name=f"x16_{ct}_{kb}"
                )
                e.dma_start(out=tl[:], in_=xs16[kb * nct + ct])
                x16_sb[(ct, kb)] = tl

            def load_ct(ct, off=0):
                for i, t in enumerate(range(ndr)):
                    load_x8(ct, t, engs[(off + i) % 3])
                for i, kb in enumerate(range(nkb)):
                    load_x16(ct, kb, engs[(off + ndr + i) % 3])

            # ct0: two packed all-K DMAs (fat contiguous runs) + weights,
            # spread so the first matmul group's deps land by ~8.5us.
            if ndr:
                x0p8_sb = x_pool.tile([C, ndr, 2, NT], f8, name="x0p8")
                tl = wt_pool.tile([C, 2, NCh], f8, tag="w8_0", name="w8_0")
                nc.sync.dma_start(out=tl[:], in_=wt8[0])
                wt8_sb[0] = tl
                nc.sync.dma_start(out=x0p8_sb[:], in_=x0p8[:])
                for t in range(ndr):
                    x8_sb[(0, t)] = x0p8_sb[:, t]
            if nkb:
                x0p16_sb = x_pool.tile([C, nkb, NT], f16, name="x0p16")
                nc.scalar.dma_start(out=x0p16_sb[:], in_=x0p16[:])
                for kb in range(nkb):
                    x16_sb[(0, kb)] = x0p16_sb[:, kb]
            for t in range(1, ndr):
                tl = wt_pool.tile([C, 2, NCh], f8, tag=f"w8_{t}", name=f"w8_{t}")
                engs[t % 3].dma_start(out=tl[:], in_=wt8[t])
                wt8_sb[t] = tl
            for kb in range(nkb):
                tl = wt_pool.tile([C, NCh], f16, tag=f"w16_{kb}", name=f"w16_{kb}")
                engs[(ndr + kb) % 3].dma_start(out=tl[:], in_=wt16[kb])
                wt16_sb[kb] = tl
            load_ct(1, off=1)

            out_engs = [getattr(nc, e) for e in OUT_ENGS]
            nmm = ndr + nkb
            di = 0
            osb_held = {}  # ob -> (osb tile, ct_of_first_half)
            for ct in range(nct):
                if ct + 2 < nct:
                    load_ct(ct + 2, off=ct)
                for ob in range(N):
                    ps = psum_pool.tile([C, NT], f32, tag="ps", name=f"ps_{ct}_{ob}")
                    c0 = ct * NT
                    mi = 0
                    for t in range(ndr):
                        nc.tensor.matmul(
                            ps[:],
                            wt8_sb[t][:, :, ob * C : (ob + 1) * C],
                            x8_sb[(ct, t)][:],
                            start=(mi == 0),
                            stop=(mi == nmm - 1),
                            perf_mode=DR,
                        )
                        mi += 1
                    for kb in range(nkb):
                        nc.tensor.matmul(
                            ps[:],
                            wt16_sb[kb][:, ob * C : (ob + 1) * C],
                            x16_sb[(ct, kb)][:],
                            start=(mi == 0),
                            stop=(mi == nmm - 1),
                        )
                        mi += 1
                    # Pair two adjacent ct halves per ob into one 2*NT-wide
                    # output DMA (half the DMA count); odd final ct flushes
                    # single-width.
                    if ob in osb_held:
                        osb, ct0h = osb_held.pop(ob)
                        nc.vector.tensor_scalar_add(
                            osb[:, NT:], ps[:], bias_sb[:, ob : ob + 1]
                        )
                        out_engs[di % len(out_engs)].dma_start(
                            out=out[ob * C : (ob + 1) * C, ct0h * NT : c0 + NT],
                            in_=osb[:],
                        )
                        di += 1
                    elif ct == nct - 1:
                        osb = out_pool.tile(
                            [C, NT], out_dt, tag="ol", name=f"ol_{ob}"
                        )
                        nc.vector.tensor_scalar_add(
                            osb[:], ps[:], bias_sb[:, ob : ob + 1]
                        )
                        out_engs[di % len(out_engs)].dma_start(
                            out=out[ob * C : (ob + 1) * C, c0 : c0 + NT],
                            in_=osb[:],
                        )
                        di += 1
                    else:
                        osb = out_pool.tile(
                            [C, 2 * NT], out_dt, tag="o", name=f"o_{ct}_{ob}"
                        )
                        nc.vector.tensor_scalar_add(
                            osb[:, :NT], ps[:], bias_sb[:, ob : ob + 1]
                        )
                        osb_held[ob] = (osb, ct)
    nc.finalize()
    return nc


def kernel(x, pos_dec, length_dec, conv_w, conv_b):
    global LAST_RESULT
    import ml_dtypes
    from concourse.bass_utils import run_bass_kernel_spmd

    f8np = ml_dtypes.float8_e4m3

    pd = np.asarray(pos_dec, dtype=np.float32)
    ld = np.asarray(length_dec, dtype=np.float32)
    Wm = np.asarray(conv_w, dtype=np.float32)
    x = np.asarray(x, dtype=np.float32).reshape(B, N, C * H * W)

    # Guard: verify the 2-way softmax saturates to [1, 0] for this input.
    # logit0 - logit1 = (1-pd)*g0 + pd*g1 - ld*((1-pd)*g1 + pd*g2) with
    # g_j = <x_n, x_{n+j mod N}>; for iid N(0,1) data g0 ~ 294912 dominates.
    g0 = np.einsum("bnd,bnd->bn", x, x)
    x1 = np.roll(x, -1, axis=1)
    g1 = np.einsum("bnd,bnd->bn", x, x1)
    g2 = np.einsum("bnd,bnd->bn", x, np.roll(x, -2, axis=1))
    l0 = (1.0 - pd) * g0 + pd * g1
    l1 = ld * ((1.0 - pd) * g1 + pd * g2)
    saturated = bool((l0 - l1).min() > 25.0)

    if saturated:
        # attn == [1, 0] exactly in fp32 -> feat_n = (1-pd_n) x_n + pd_n x_{n+1};
        # fold the interpolation into the conv weights host-side.
        W_eff = np.empty_like(Wm)
        for m in range(N):
            pm = (m - 1) % N
            W_eff[:, m * C : (m + 1) * C] = \
                (1.0 - pd[m]) * Wm[:, m * C : (m + 1) * C] + \
                pd[pm] * Wm[:, pm * C : (pm + 1) * C]
        feed = x
    else:
        # General path: materialize feat with the true attention weights on
        # the host; same device kernel with the plain conv weights.
        gap = l1 - l0
        a1 = 1.0 / (1.0 + np.exp(np.clip(-gap, -87.0, 87.0)))
        a0 = 1.0 - a1
        c0 = (a0 * (1.0 - pd))[:, :, None]
        c1 = (a0 * pd + a1 * ld * (1.0 - pd))[:, :, None]
        c2 = (a1 * ld * pd)[:, :, None]
        feed = c0 * x + c1 * x1 + c2 * np.roll(x, -2, axis=1)
        W_eff = Wm

    fp8_rows = FP8_ROWS
    ndr = fp8_rows // 256
    nkb = (NCh - fp8_rows) // 128

    WT = np.ascontiguousarray(W_eff.T)  # [c_in, o]
    wt8 = np.ascontiguousarray(
        WT[:fp8_rows].reshape(ndr, 2, C, NCh).transpose(0, 2, 1, 3).astype(f8np)
    ) if ndr else None
    wt16 = np.ascontiguousarray(
        WT[fp8_rows:].reshape(nkb, C, NCh).astype(np.float16)
    ) if nkb else None
    bias_t = np.ascontiguousarray(
        np.asarray(conv_b, dtype=np.float32).reshape(N, C).T
    )  # [C, N]

    feed = feed.reshape(B, NCh, HW)
    nct = COLS // NT
    in_maps = []
    for c in range(NCORES):
        Xc = np.concatenate([feed[2 * c], feed[2 * c + 1]], axis=1)  # [NCh, COLS]
        m = {"bias": bias_t}
        if ndr:
            x8 = Xc[:fp8_rows].astype(f8np)  # [fp8_rows, COLS]
            # [t*nct+ct, p, j, n] = x8[256t + 128j + p, ct*NT + n]
            v = x8.reshape(ndr, 2, C, nct, NT)
            m["xs8"] = np.ascontiguousarray(
                v.transpose(0, 3, 2, 1, 4).reshape(ndr * nct, C, 2, NT)
            )
            # [p, t, j, n] = x8[256t + 128j + p, n]
            m["x0p8"] = np.ascontiguousarray(v[:, :, :, 0].transpose(2, 0, 1, 3))
            m["wt8"] = wt8
        if nkb:
            x16 = Xc[fp8_rows:].astype(np.float16)  # [NCh-fp8_rows, COLS]
            v = x16.reshape(nkb, C, nct, NT)
            m["xs16"] = np.ascontiguousarray(
                v.transpose(0, 2, 1, 3).reshape(nkb * nct, C, NT)
            )
            m["x0p16"] = np.ascontiguousarray(v[:, :, 0].transpose(1, 0, 2))
            m["wt16"] = wt16
        in_maps.append(m)

    key = (fp8_rows, NT, OUT_DTYPE, PS_BUFS, OSB_BUFS, WARMUP_MMS, OUT_ENGS)
    if _cache.get("key") != key:
        _cache["nc"] = _build_nc(fp8_rows)
        _cache["key"] = key
    nc = _cache["nc"]

    res = None
    for attempt in range(3):
        try:
            res = run_bass_kernel_spmd(
                nc, in_maps, core_ids=list(range(NCORES)), trace=TRACE,
                trace_cores=TRACE_CORES,
            )
            break
        except Exception:
            # The PJRT/axon dispatch occasionally hits a transient
            # device-unrecoverable error; a retry re-initializes and succeeds.
            if attempt == 2:
                raise
            import time

            time.sleep(2.0)
    LAST_RESULT = res

    out = np.empty((B, NCh, HW), dtype=np.float32)
    for c in range(NCORES):
        oc = np.asarray(res.results[c]["out"], dtype=np.float32)  # [NCh, COLS]
        out[2 * c] = oc[:, :HW]
        out[2 * c + 1] = oc[:, HW:]
    out += x.reshape(B, NCh, HW)  # residual (identity) added exactly in fp32
    return out.reshape(B, NCh, H, W)
